# revision 26
# baseline (speedup 1.0000x reference)
"""Causal self-attention (B=2, T=2048, D=1024, H=16, dh=64) on 8 TRN2
NeuronCores.

Sharding: 2-way batch parallel x 4-way head (tensor) parallel.
Core c: batch b = c // 4, heads 4*(c%4) .. 4*(c%4)+3.

Per-core program (Megatron-style TP, bf16 matmuls, f32 softmax stats):
  QT = Wq_s.T @ x_b.T          [256, 2048]  (head dims on partitions)
  KT = Wk_s.T @ x_b.T          [256, 2048]
  V  = x_b @ Wv_s              [2048, 256]  (per head: [V|1] even, [1|V] odd)
  per head pair m, key tile jt, 512-wide query chunk c:
      ST(hb) = Kh_jt @ Qh_c^T for hb=0,1 into the two 512-col halves of one
           [128,1024] PSUM tile (the K=64 pair row-tiles into the PE array
           concurrently); diagonal tiles restricted to live columns, with one
           3D-AP mask add covering both halves' [128,128] diagonal blocks
      PT = exp(ST/8) as ONE wide ACTIVATE over both halves (3D AP) --
           ScalarE has ~300ns fixed cost per op, so halving op count
           relieves the S->exp->AV chain
      [OT; L] = [Vh | 1]^T @ PT   accumulated over key tiles in PSUM
      OTn = OT * (1/L broadcast along partitions); 1/L is a single
      DVE reciprocal_approx_fast reading the L row straight from PSUM
  Ypart = OTn.T @ Wo_s         [2048, 1024] partial over heads
  ReduceScatter(add) in 4 pieces of [512, 1024] directly into the
  ExternalOutput tensors, each launched as soon as its Y tiles land.

All non-psO PSUM users (S, Q|K, V, Y, norm broadcast) share ONE
[128,1024] 2-bank pool (bufs=2) so PSUM fits: 2*2 + psO 2*2 = 8 banks.

Input DMAs are column-blocked (wqk, then x cols 0:512, wv, x 512:2048) so
the first projection matmuls start ~6us in instead of ~13us.
"""

import numpy as np

import concourse.bass as bass
import concourse.mybir as mybir
import concourse.tile as tile
from concourse import bacc
from concourse.bass_utils import run_bass_kernel_spmd

P = 128          # partitions
T = 2048         # tokens
D = 1024         # d_model
H_LOC = 4        # heads per core
DH = 64          # head dim
DHC = H_LOC * DH  # 256 head-dim cols per core
KD = D // P      # 8 k-tiles over d_model
NT = T // P      # 16 token tiles
IC = 512         # query chunk width
NIC = T // IC    # 4
SCALE = 1.0 / np.sqrt(DH)
MASKVAL = -1e9
RS_ROWS = [512, 512, 512, 256, 256]
RS_START = [0, 512, 1024, 1536, 1792]
NP_RS = len(RS_ROWS)

f32 = mybir.dt.float32
bf16 = mybir.dt.bfloat16
f16 = mybir.dt.float16

N_CORES = 8
GROUPS = [[0, 1, 2, 3], [4, 5, 6, 7]]


def build_nc(dbg=False):
    nc = bacc.Bacc("TRN2", target_bir_lowering=False, debug=False,
                   num_devices=N_CORES)

    xT_d = nc.dram_tensor("xT", [D, T], bf16, kind="ExternalInput")
    wqkv_d = nc.dram_tensor("wqkv", [D, 3 * DHC], bf16, kind="ExternalInput")
    wo_d = nc.dram_tensor("wo", [DHC, D], bf16, kind="ExternalInput")
    cm_d = nc.dram_tensor("cmask", [P, P], f32, kind="ExternalInput")
    ones_d = nc.dram_tensor("ones", [P, DH], bf16, kind="ExternalInput")
    out_d = [nc.dram_tensor(f"out{p}", [RS_ROWS[p] // 4, D], f16,
                            kind="ExternalOutput") for p in range(NP_RS)]
    rs_out = None

    with tile.TileContext(nc) as tc:
        with (
            tc.tile_pool(name="persist", bufs=1) as persist,
            tc.tile_pool(name="work", bufs=1) as work,
            tc.tile_pool(name="psum", bufs=1, space="PSUM") as psum,
            tc.tile_pool(name="dram", bufs=1, space="DRAM") as dram,
        ):
            # ---- persistent SBUF tensors ----
            wo2 = persist.tile([P, 2, D], bf16, name="wot")
            wo_t = [wo2[:, m] for m in range(2)]
            # doubled causal mask: [128, 2, 128] so ONE 3D tensor_tensor
            # masks both heads' diagonal blocks of the merged psS tile
            cmask2 = persist.tile([P, 2, P], f32)
            ones_sb = persist.tile([P, NT * H_LOC], bf16)
            ones64 = persist.tile([DH + 1, DH], bf16)

            def emit_early_persist_dmas():
                # before the 3MB x tail: cmask2/ones gate the first chunk's
                # mask add and the va ones fill
                for r in range(2):
                    nc.sync.dma_start(out=cmask2[:, r, :], in_=cm_d[:])
                nc.sync.dma_start(out=ones_sb[:], in_=ones_d[:, :])
                nc.sync.dma_start(out=ones64[DH:DH + 1, :],
                                  in_=ones_d[DH:DH + 1, :])

            def emit_wo_dma():
                nc.sync.dma_start(
                    out=wo2[:],
                    in_=wo_d[:].rearrange("(m p) c -> p m c", m=2))
            qt, kt = [], []
            for m in range(2):
                qt.append(persist.tile([P, T], bf16, name=f"qt{m}"))
                kt.append(persist.tile([P, T], bf16, name=f"kt{m}"))
            # va_all[:, tt, h, :]: [Vh(64) | 1] = 65 cols per (tile, head);
            # one big tile so the ones columns fill with a single DVE copy
            va_all = persist.tile([P, NT, H_LOC, DH + 1], bf16, name="va")
            va = [va_all[:, tt] for tt in range(NT)]
            otn = [persist.tile([P, T], bf16, name=f"otn{m}") for m in range(2)]

            ybounce = dram.tile([T, D], f16, name="ybounce")
            rs_out = [dram.tile([RS_ROWS[p] // 4, D], f16, name=f"rs_out{p}")
                      for p in range(NP_RS)]

            def big_ps(name):
                return psum.tile([P, 2 * IC], f32, tag="big", name=name,
                                 bufs=2)

            # ---- emission helpers ----
            def qtkt_unit(m, ic, wq_t, wk_t, xt):
                def go():
                    ps = big_ps("psQK")
                    psQ = ps[:, 0:IC]
                    psK = ps[:, IC:2 * IC]
                    for k in range(KD):
                        nc.tensor.matmul(
                            psQ, wq_t[k][:, m * P:(m + 1) * P],
                            xt[k][:, ic * IC:(ic + 1) * IC],
                            start=(k == 0), stop=(k == KD - 1))
                        nc.tensor.matmul(
                            psK, wk_t[k][:, m * P:(m + 1) * P],
                            xt[k][:, ic * IC:(ic + 1) * IC],
                            start=(k == 0), stop=(k == KD - 1))
                    nc.vector.tensor_copy(qt[m][:, ic * IC:(ic + 1) * IC],
                                          psQ)
                    nc.vector.tensor_copy(kt[m][:, ic * IC:(ic + 1) * IC],
                                          psK)
                return go

            def ones_fill():
                # single DVE copy filling ALL va tiles' ones columns
                nc.vector.tensor_copy(
                    va_all.rearrange("p t h c -> p (t h) c")[:, :, DH:DH + 1],
                    ones_sb[:].unsqueeze(-1))

            def v_unit(tt, wv_t, xt):
                def go():
                    psV = big_ps("psV")[:, 0:DHC]
                    for k in range(KD):
                        nc.tensor.matmul(
                            psV, xt[k][:, tt * P:(tt + 1) * P], wv_t[k][:],
                            start=(k == 0), stop=(k == KD - 1))
                    psV3 = psV.rearrange("p (h c) -> p h c", c=DH)
                    nc.vector.tensor_copy(va[tt][:, :, 0:DH], psV3[:, :, :])
                return go

            def emit_normalize(m, q0, psO_e, psO_o, qw=IC):
                # OTn[d, i] = OT[d, i] * (1/L[i]); PE broadcast of the L row
                # (see baseline docstring) -- engines are lane-locked.
                w = qw
                cc = slice(q0, q0 + qw)
                rr = slice(0, qw)
                lre = work.tile([DH + 1, IC], bf16, tag="lre", name="lre",
                                bufs=2)
                lro = work.tile([DH + 1, IC], bf16, tag="lro", name="lro",
                                bufs=2)
                nc.vector.tensor_copy(lre[DH:DH + 1, rr], psO_e[DH:DH + 1, rr])
                nc.vector.tensor_copy(lro[DH:DH + 1, rr], psO_o[DH:DH + 1, rr])
                psB = big_ps("psB")
                psBe = psB[0:DH, 0:IC]
                psBo = psB[0:DH, IC:2 * IC]
                nc.tensor.matmul(psBe[:, rr], ones64[DH:DH + 1, :],
                                 lre[DH:DH + 1, rr], start=True, stop=True)
                nc.tensor.matmul(psBo[:, rr], ones64[DH:DH + 1, :],
                                 lro[DH:DH + 1, rr], start=True, stop=True)
                bc = work.tile([DH, 2 * IC], f32, tag="bc", name="bc", bufs=2)
                bc3 = bc.rearrange("p (b q) -> p b q", b=2)
                psB3 = psB.rearrange("p (b q) -> p b q", b=2)
                nc.vector.reciprocal_approx_fast(bc3[:, :, rr],
                                                 psB3[0:DH, :, rr])
                bce = bc[:, 0:IC]
                bco = bc[:, IC:2 * IC]
                nc.vector.tensor_tensor(
                    out=otn[m][0:DH, cc], in0=psO_e[0:DH, rr], in1=bce[:, rr],
                    op=mybir.AluOpType.mult)
                otmp = work.tile([DH, IC], bf16, tag="otmp", name="otmp",
                                 bufs=2)
                nc.vector.tensor_tensor(
                    out=otmp[:, 0:w], in0=psO_o[0:DH, rr], in1=bco[:, rr],
                    op=mybir.AluOpType.mult)
                nc.sync.dma_start(out=otn[m][DH:P, cc], in_=otmp[:, 0:w])

            # software-pipelined emission: the O-matmuls (and the normalize
            # at a chunk's last key tile) trail the S/exp stream by 2
            # chain-pairs so the PE queue never stalls behind an exp wait.
            tail = []

            def drain(n):
                while len(tail) > n:
                    tail.pop(0)()

            def emit_qrange(q0, qw, m, filler, pops=1):
                # attention for queries [q0, q0+qw), head pair m
                psO = {}
                njt = (q0 + qw) // P
                jd = q0 // P  # first diagonal key tile
                for hb in range(2):
                    psO[hb] = psum.tile([DH + 1, IC], f32,
                                        tag=f"psO{hb}",
                                        name=f"psO{hb}", bufs=2)
                for jt in range(njt):
                    diag = jt >= jd
                    col0 = jt * P - q0 if diag else 0
                    qs = slice(q0 + col0, q0 + qw)
                    psS = big_ps("psS")
                    psS3 = psS.rearrange("p (b q) -> p b q", b=2)
                    for hb in range(2):
                        po = hb * DH
                        nc.tensor.matmul(
                            psS[:, hb * IC + col0:hb * IC + qw],
                            kt[m][po:po + DH, jt * P:(jt + 1) * P],
                            qt[m][po:po + DH, qs],
                            start=True, stop=True)
                    if diag:
                        nc.vector.tensor_tensor(
                            out=psS3[:, :, col0:col0 + P],
                            in0=psS3[:, :, col0:col0 + P],
                            in1=cmask2[:],
                            op=mybir.AluOpType.add)
                    pt = work.tile([P, 2 * IC], bf16, tag="pt",
                                   name="pt", bufs=4)
                    pt3 = pt.rearrange("p (b q) -> p b q", b=2)
                    nc.scalar.activation(
                        pt3[:, :, col0:qw], psS3[:, :, col0:qw],
                        mybir.ActivationFunctionType.Exp,
                        scale=SCALE)

                    def o_mm(m=m, q0=q0, qw=qw, jt=jt, col0=col0,
                             psO=dict(psO), njt=njt, pt=pt):
                        for hb in range(2):
                            h = 2 * m + hb
                            nc.tensor.matmul(
                                psO[hb][0:DH + 1, slice(col0, qw)],
                                va[jt][:, h],
                                pt[:, hb * IC + col0:hb * IC + qw],
                                start=(jt == 0), stop=(jt == njt - 1))
                        if jt == njt - 1:
                            emit_normalize(m, q0, psO[0], psO[1], qw=qw)
                    tail.append(o_mm)
                    drain(2)
                    if jt >= 2:
                        for _ in range(pops):
                            if filler:
                                filler.pop(0)()

            def y_unit(tt):
                def go():
                    ysb = work.tile([P, D], f16, tag="ysb", name="ysb",
                                    bufs=3)
                    psY = big_ps("psY")
                    for ncol in range(2):
                        for m in range(2):
                            nc.tensor.matmul(
                                psY[:, ncol * IC:(ncol + 1) * IC],
                                otn[m][:, tt * P:(tt + 1) * P],
                                wo_t[m][:, ncol * IC:(ncol + 1) * IC],
                                start=(m == 0), stop=(m == 1))
                    nc.vector.tensor_copy(ysb[:], psY[:])
                    nc.sync.dma_start(
                        out=ybounce[tt * P:(tt + 1) * P, :], in_=ysb[:])
                return go

            def rs_unit(p):
                def go():
                    nc.gpsimd.collective_compute(
                        "ReduceScatter",
                        mybir.AluOpType.add,
                        replica_groups=GROUPS,
                        ins=[ybounce[RS_START[p]:RS_START[p] + RS_ROWS[p], :]],
                        outs=[rs_out[p][:]],
                    )
                    # drain on the gpsimd queue: it blocks behind the
                    # collective anyway, so this issues right after RS p
                    # completes without head-blocking the sync queue
                    nc.gpsimd.dma_start(out=out_d[p][:], in_=rs_out[p][:])
                return go

            # ---- program ----
            # Warm the CC path during the input-DMA ramp.
            warm_in = dram.tile([4, D], f16, name="warm_in")
            warm_out = dram.tile([1, D], f16, name="warm_out")
            nc.gpsimd.collective_compute(
                "ReduceScatter", mybir.AluOpType.add, replica_groups=GROUPS,
                ins=[warm_in[:]], outs=[warm_out[:]])

            with tc.tile_pool(name="qkv_in", bufs=1) as qkv_in:
                xt = []
                wq_t, wk_t, wv_t = [], [], []
                wqkv = qkv_in.tile([P, KD, 3 * DHC], bf16, name="wqkv")
                for k in range(KD):
                    xt.append(qkv_in.tile([P, T], bf16, name=f"xt{k}"))
                    wq_t.append(wqkv[:, k, 0:DHC])
                    wk_t.append(wqkv[:, k, DHC:2 * DHC])
                    wv_t.append(wqkv[:, k, 2 * DHC:3 * DHC])
                # column-blocked loads: everything the first qtkt/v units
                # need (wq|wk, x cols 0:512, wv) lands in the first ~2.5MB,
                # persist tensors (cmask/ones/wo) before the 3MB x tail.
                # k-pairs merged per DMA to halve the ~650ns/DMA issue cost.
                for k in range(0, KD, 2):
                    nc.sync.dma_start(
                        out=wqkv[:, k:k + 2, 0:2 * DHC],
                        in_=wqkv_d[k * P:(k + 2) * P, 0:2 * DHC]
                        .rearrange("(j p) c -> p j c", j=2))
                    nc.sync.dma_start(out=xt[k][:, 0:IC],
                                      in_=xT_d[k * P:(k + 1) * P, 0:IC])
                    nc.sync.dma_start(out=xt[k + 1][:, 0:IC],
                                      in_=xT_d[(k + 1) * P:(k + 2) * P, 0:IC])
                emit_early_persist_dmas()
                for k in range(0, KD, 2):
                    nc.sync.dma_start(
                        out=wqkv[:, k:k + 2, 2 * DHC:3 * DHC],
                        in_=wqkv_d[k * P:(k + 2) * P, 2 * DHC:3 * DHC]
                        .rearrange("(j p) c -> p j c", j=2))
                emit_wo_dma()
                for k in range(KD):
                    nc.sync.dma_start(out=xt[k][:, IC:T],
                                      in_=xT_d[k * P:(k + 1) * P, IC:T])
                ones_fill()

                # preload the ScalarE EXP table off the critical path (the
                # first ACTIVATE otherwise pays a ~1.3us ACT_TABLE_LOAD at
                # the start of attention)
                warm_act = work.tile([1, 2], f32, name="warm_act")
                nc.vector.memset(warm_act[:], 0.0)
                nc.scalar.activation(warm_act[:], warm_act[:],
                                     mybir.ActivationFunctionType.Exp,
                                     scale=SCALE)

                qtkt_unit(0, 0, wq_t, wk_t, xt)()
                qtkt_unit(1, 0, wq_t, wk_t, xt)()
                for tt in range(4):
                    v_unit(tt, wv_t, xt)()
                fillers = {
                    0: [qtkt_unit(0, 1, wq_t, wk_t, xt),
                        qtkt_unit(1, 1, wq_t, wk_t, xt)]
                       + [v_unit(tt, wv_t, xt) for tt in range(4, 8)],
                    1: [y_unit(tt) for tt in range(0, 4)] + [rs_unit(0)]
                       + [qtkt_unit(0, 2, wq_t, wk_t, xt),
                          qtkt_unit(1, 2, wq_t, wk_t, xt)]
                       + [v_unit(tt, wv_t, xt) for tt in range(8, 12)],
                    2: [y_unit(tt) for tt in range(4, 8)] + [rs_unit(1)]
                       + [qtkt_unit(0, 3, wq_t, wk_t, xt),
                          qtkt_unit(1, 3, wq_t, wk_t, xt)]
                       + [v_unit(tt, wv_t, xt) for tt in range(12, 16)],
                    3: [y_unit(tt) for tt in range(8, 12)] + [rs_unit(2)],
                }
                for c in range(NIC - 1):
                    f = fillers[c]
                    emit_qrange(c * IC, IC, 0, f, pops=2)
                    emit_qrange(c * IC, IC, 1, f, pops=2)
                    drain(0)
                    for g in f:
                        g()
                    f.clear()
                # last chunk as two 256-wide passes: rows 1536..1792 finish
                # ~25us before the end, so their RS piece overlaps the
                # second pass and only a 256-row piece is exposed at the tail
                f = fillers[NIC - 1]
                emit_qrange(3 * IC, IC // 2, 0, f, pops=2)
                emit_qrange(3 * IC, IC // 2, 1, f, pops=2)
                drain(0)
                for g in f:
                    g()
                f.clear()
                fl = [y_unit(12), y_unit(13), rs_unit(3)]
                emit_qrange(3 * IC + IC // 2, IC // 2, 0, fl, pops=1)
                emit_qrange(3 * IC + IC // 2, IC // 2, 1, fl, pops=1)
                drain(0)
                for g in fl:
                    g()
            for f in [y_unit(14), y_unit(15), rs_unit(4)]:
                f()

    nc.compile()
    return nc


def make_cmask():
    """[128, 128] triangular diagonal-block mask: key j (partition), query i
    (free col, relative to the key tile start): valid (0.0) iff j <= i."""
    j = np.arange(P)[:, None]
    i = np.arange(P)[None, :]
    return np.where(j <= i, 0.0, MASKVAL).astype(np.float32)


def shard_inputs(x, Wq, Wk, Wv, Wo):
    import ml_dtypes
    bf = ml_dtypes.bfloat16
    cmask = make_cmask()
    in_maps = []
    for c in range(N_CORES):
        b, r = divmod(c, 4)
        sl = slice(r * DHC, (r + 1) * DHC)
        wqkv = np.concatenate([Wq[:, sl], Wk[:, sl], Wv[:, sl]], axis=1)
        in_maps.append({
            "xT": np.ascontiguousarray(x[b].T).astype(bf),
            "wqkv": np.ascontiguousarray(wqkv).astype(bf),
            "wo": np.ascontiguousarray(Wo[sl, :]).astype(bf),
            "cmask": cmask,
            "ones": np.ones((P, DH), dtype=bf),
        })
    return in_maps


def assemble(results, B=2):
    out = np.empty((B, T, D), dtype=np.float32)
    for c in range(N_CORES):
        b, r = divmod(c, 4)
        for p in range(NP_RS):
            res = results[c][f"out{p}"].astype(np.float32)
            rows = RS_ROWS[p] // 4
            out[b, RS_START[p] + r * rows: RS_START[p] + (r + 1) * rows, :] \
                = res
    return out


_NC_CACHE = None


def get_nc():
    global _NC_CACHE
    if _NC_CACHE is None:
        _NC_CACHE = build_nc()
    return _NC_CACHE


def run(inputs, trace=False):
    nc = get_nc()
    in_maps = shard_inputs(inputs["x"], inputs["Wq"], inputs["Wk"],
                           inputs["Wv"], inputs["Wo"])
    res = run_bass_kernel_spmd(nc, in_maps, core_ids=list(range(N_CORES)),
                               trace=trace)
    return assemble(res.results), res


def kernel(x, Wq, Wk, Wv, Wo):
    out, _ = run({"x": np.asarray(x), "Wq": np.asarray(Wq),
                  "Wk": np.asarray(Wk), "Wv": np.asarray(Wv),
                  "Wo": np.asarray(Wo)})
    return out


# revision 33
# speedup vs baseline: 1.2220x; 1.2220x over previous
"""Causal self-attention (B=2, T=2048, D=1024, H=16, dh=64) on 8 TRN2
NeuronCores.

Sharding: 2-way batch parallel x 4-way head (tensor) parallel.
Core c: batch b = c // 4, heads 4*(c%4) .. 4*(c%4)+3.

Per-core program (Megatron-style TP, bf16 matmuls, f32 softmax stats):
  QT = Wq_s.T @ x_b.T          [256, 2048]  (head dims on partitions)
  KT = Wk_s.T @ x_b.T          [256, 2048]
  V  = x_b @ Wv_s              [2048, 256]  (per head: [V|1] even, [1|V] odd)
  per head pair m, key tile jt, 512-wide query chunk c:
      ST(hb) = Kh_jt @ Qh_c^T for hb=0,1 into the two 512-col halves of one
           [128,1024] PSUM tile (the K=64 pair row-tiles into the PE array
           concurrently); diagonal tiles restricted to live columns, with one
           3D-AP mask add covering both halves' [128,128] diagonal blocks
      PT = exp(ST/8) as ONE wide ACTIVATE over both halves (3D AP) --
           ScalarE has ~300ns fixed cost per op, so halving op count
           relieves the S->exp->AV chain
      [OT; L] = [Vh | 1]^T @ PT   accumulated over key tiles in PSUM
      OTn = OT * (1/L broadcast along partitions); 1/L is a single
      DVE reciprocal_approx_fast reading the L row straight from PSUM
  Ypart = OTn.T @ Wo_s         [2048, 1024] partial over heads
  ReduceScatter(add) in 4 pieces of [512, 1024] directly into the
  ExternalOutput tensors, each launched as soon as its Y tiles land.

All non-psO PSUM users (S, Q|K, V, Y, norm broadcast) share ONE
[128,1024] 2-bank pool (bufs=2) so PSUM fits: 2*2 + psO 2*2 = 8 banks.

Input DMAs are column-blocked (wqk, then x cols 0:512, wv, x 512:2048) so
the first projection matmuls start ~6us in instead of ~13us.
"""

import numpy as np

import concourse.bass as bass
import concourse.mybir as mybir
import concourse.tile as tile
from concourse import bacc
from concourse.bass_utils import run_bass_kernel_spmd

P = 128          # partitions
T = 2048         # tokens
D = 1024         # d_model
H_LOC = 4        # heads per core
DH = 64          # head dim
DHC = H_LOC * DH  # 256 head-dim cols per core
KD = D // P      # 8 k-tiles over d_model
NT = T // P      # 16 token tiles
IC = 512         # query chunk width
NIC = T // IC    # 4
SCALE = 1.0 / np.sqrt(DH)
MASKVAL = -1e9
RS_ROWS = [512, 512, 512, 256, 256]
RS_START = [0, 512, 1024, 1536, 1792]
NP_RS = len(RS_ROWS)

f32 = mybir.dt.float32
bf16 = mybir.dt.bfloat16
f16 = mybir.dt.float16

N_CORES = 8
GROUPS = [[0, 1, 2, 3], [4, 5, 6, 7]]
# core c: batch CORE_B[c], head-quad CORE_R[c]; every group must hold the
# 4 cores of one batch
CORE_B = [c // 4 for c in range(N_CORES)]
CORE_R = [c % 4 for c in range(N_CORES)]


def build_nc(dbg=False, groups=None):
    groups = groups if groups is not None else GROUPS
    nc = bacc.Bacc("TRN2", target_bir_lowering=False, debug=False,
                   num_devices=N_CORES)

    xT_d = nc.dram_tensor("xT", [D, T], bf16, kind="ExternalInput")
    wqkv_d = nc.dram_tensor("wqkv", [D, 3 * DHC], bf16, kind="ExternalInput")
    wo_d = nc.dram_tensor("wo", [DHC, D], bf16, kind="ExternalInput")
    cm_d = nc.dram_tensor("cmask", [P, P], f32, kind="ExternalInput")
    ones_d = nc.dram_tensor("ones", [P, DH], bf16, kind="ExternalInput")
    out_d = [nc.dram_tensor(f"out{p}", [RS_ROWS[p] // 4, D], f16,
                            kind="ExternalOutput") for p in range(NP_RS)]
    rs_out = None

    with tile.TileContext(nc) as tc:
        with (
            tc.tile_pool(name="persist", bufs=1) as persist,
            tc.tile_pool(name="work", bufs=1) as work,
            tc.tile_pool(name="psum", bufs=1, space="PSUM") as psum,
            tc.tile_pool(name="dram", bufs=1, space="DRAM") as dram,
        ):
            # ---- persistent SBUF tensors ----
            wo2 = persist.tile([P, 2, D], bf16, name="wot")
            wo_t = [wo2[:, m] for m in range(2)]
            # doubled causal mask: [128, 2, 128] so ONE 3D tensor_tensor
            # masks both heads' diagonal blocks of the merged psS tile
            cmask2 = persist.tile([P, 2, P], f32)
            ones_sb = persist.tile([P, NT * H_LOC], bf16)
            ones64 = persist.tile([DH + 1, DH], bf16)

            def emit_early_persist_dmas():
                # before the 3MB x tail: cmask2/ones gate the first chunk's
                # mask add and the va ones fill
                for r in range(2):
                    nc.sync.dma_start(out=cmask2[:, r, :], in_=cm_d[:])
                nc.sync.dma_start(out=ones_sb[:], in_=ones_d[:, :])
                nc.sync.dma_start(out=ones64[DH:DH + 1, :],
                                  in_=ones_d[DH:DH + 1, :])

            def emit_wo_dma():
                nc.sync.dma_start(
                    out=wo2[:],
                    in_=wo_d[:].rearrange("(m p) c -> p m c", m=2))
            qt, kt = [], []
            for m in range(2):
                qt.append(persist.tile([P, T], bf16, name=f"qt{m}"))
                kt.append(persist.tile([P, T], bf16, name=f"kt{m}"))
            # va_all[:, tt, h, :]: [Vh(64) | 1] = 65 cols per (tile, head);
            # one big tile so the ones columns fill with a single DVE copy
            va_all = persist.tile([P, NT, H_LOC, DH + 1], bf16, name="va")
            va = [va_all[:, tt] for tt in range(NT)]
            otn = [persist.tile([P, T], bf16, name=f"otn{m}") for m in range(2)]

            ybounce = dram.tile([T, D], f16, name="ybounce")
            rs_out = [dram.tile([RS_ROWS[p] // 4, D], f16, name=f"rs_out{p}")
                      for p in range(NP_RS)]

            def big_ps(name):
                # 3 bufs (6 banks) + 2 psO (1 bank each) = 8 banks: the
                # deeper rotation lets S(j+3) issue before exp(j) retires,
                # absorbing ScalarE jitter in the S->exp->AV chain
                return psum.tile([P, 2 * IC], f32, tag="big", name=name,
                                 bufs=3)

            # ---- emission helpers ----
            def qtkt_unit(m, ic, wq_t, wk_t, xt):
                def go():
                    ps = big_ps("psQK")
                    psQ = ps[:, 0:IC]
                    psK = ps[:, IC:2 * IC]
                    for k in range(KD):
                        nc.tensor.matmul(
                            psQ, wq_t[k][:, m * P:(m + 1) * P],
                            xt[k][:, ic * IC:(ic + 1) * IC],
                            start=(k == 0), stop=(k == KD - 1))
                        nc.tensor.matmul(
                            psK, wk_t[k][:, m * P:(m + 1) * P],
                            xt[k][:, ic * IC:(ic + 1) * IC],
                            start=(k == 0), stop=(k == KD - 1))
                    nc.vector.tensor_copy(qt[m][:, ic * IC:(ic + 1) * IC],
                                          psQ)
                    nc.vector.tensor_copy(kt[m][:, ic * IC:(ic + 1) * IC],
                                          psK)
                return go

            def ones_fill():
                # single DVE copy filling ALL va tiles' ones columns
                nc.vector.tensor_copy(
                    va_all.rearrange("p t h c -> p (t h) c")[:, :, DH:DH + 1],
                    ones_sb[:].unsqueeze(-1))

            def v_unit(tt, wv_t, xt):
                def go():
                    psV = big_ps("psV")[:, 0:DHC]
                    for k in range(KD):
                        nc.tensor.matmul(
                            psV, xt[k][:, tt * P:(tt + 1) * P], wv_t[k][:],
                            start=(k == 0), stop=(k == KD - 1))
                    psV3 = psV.rearrange("p (h c) -> p h c", c=DH)
                    nc.vector.tensor_copy(va[tt][:, :, 0:DH], psV3[:, :, :])
                return go

            def emit_normalize(m, q0, psO_e, psO_o, qw=IC):
                # OTn[d, i] = OT[d, i] * (1/L[i]); PE broadcast of the L row
                # (see baseline docstring) -- engines are lane-locked.
                w = qw
                cc = slice(q0, q0 + qw)
                rr = slice(0, qw)
                lre = work.tile([DH + 1, IC], bf16, tag="lre", name="lre",
                                bufs=2)
                lro = work.tile([DH + 1, IC], bf16, tag="lro", name="lro",
                                bufs=2)
                nc.vector.tensor_copy(lre[DH:DH + 1, rr], psO_e[DH:DH + 1, rr])
                nc.vector.tensor_copy(lro[DH:DH + 1, rr], psO_o[DH:DH + 1, rr])
                psB = big_ps("psB")
                psBe = psB[0:DH, 0:IC]
                psBo = psB[0:DH, IC:2 * IC]
                nc.tensor.matmul(psBe[:, rr], ones64[DH:DH + 1, :],
                                 lre[DH:DH + 1, rr], start=True, stop=True)
                nc.tensor.matmul(psBo[:, rr], ones64[DH:DH + 1, :],
                                 lro[DH:DH + 1, rr], start=True, stop=True)
                bc = work.tile([DH, 2 * IC], f32, tag="bc", name="bc", bufs=2)
                bc3 = bc.rearrange("p (b q) -> p b q", b=2)
                psB3 = psB.rearrange("p (b q) -> p b q", b=2)
                nc.vector.reciprocal_approx_fast(bc3[:, :, rr],
                                                 psB3[0:DH, :, rr])
                bce = bc[:, 0:IC]
                bco = bc[:, IC:2 * IC]
                nc.vector.tensor_tensor(
                    out=otn[m][0:DH, cc], in0=psO_e[0:DH, rr], in1=bce[:, rr],
                    op=mybir.AluOpType.mult)
                otmp = work.tile([DH, IC], bf16, tag="otmp", name="otmp",
                                 bufs=2)
                nc.vector.tensor_tensor(
                    out=otmp[:, 0:w], in0=psO_o[0:DH, rr], in1=bco[:, rr],
                    op=mybir.AluOpType.mult)
                nc.sync.dma_start(out=otn[m][DH:P, cc], in_=otmp[:, 0:w])

            # software-pipelined emission: the O-matmuls (and the normalize
            # at a chunk's last key tile) trail the S/exp stream by 2
            # chain-pairs so the PE queue never stalls behind an exp wait.
            tail = []

            def drain(n):
                while len(tail) > n:
                    tail.pop(0)()

            def emit_qrange(q0, qw, m, filler, pops=1):
                # attention for queries [q0, q0+qw), head pair m
                psO = {}
                njt = (q0 + qw) // P
                jd = q0 // P  # first diagonal key tile
                for hb in range(2):
                    psO[hb] = psum.tile([DH + 1, IC], f32,
                                        tag=f"psO{hb}",
                                        name=f"psO{hb}", bufs=1)
                for jt in range(njt):
                    diag = jt >= jd
                    col0 = jt * P - q0 if diag else 0
                    qs = slice(q0 + col0, q0 + qw)
                    psS = big_ps("psS")
                    psS3 = psS.rearrange("p (b q) -> p b q", b=2)
                    for hb in range(2):
                        po = hb * DH
                        nc.tensor.matmul(
                            psS[:, hb * IC + col0:hb * IC + qw],
                            kt[m][po:po + DH, jt * P:(jt + 1) * P],
                            qt[m][po:po + DH, qs],
                            start=True, stop=True)
                    if diag:
                        nc.vector.tensor_tensor(
                            out=psS3[:, :, col0:col0 + P],
                            in0=psS3[:, :, col0:col0 + P],
                            in1=cmask2[:],
                            op=mybir.AluOpType.add)
                    pt = work.tile([P, 2 * IC], bf16, tag="pt",
                                   name="pt", bufs=4)
                    pt3 = pt.rearrange("p (b q) -> p b q", b=2)
                    nc.scalar.activation(
                        pt3[:, :, col0:qw], psS3[:, :, col0:qw],
                        mybir.ActivationFunctionType.Exp,
                        scale=SCALE)

                    def o_mm(m=m, q0=q0, qw=qw, jt=jt, col0=col0,
                             psO=dict(psO), njt=njt, pt=pt):
                        for hb in range(2):
                            h = 2 * m + hb
                            nc.tensor.matmul(
                                psO[hb][0:DH + 1, slice(col0, qw)],
                                va[jt][:, h],
                                pt[:, hb * IC + col0:hb * IC + qw],
                                start=(jt == 0), stop=(jt == njt - 1))
                        if jt == njt - 1:
                            emit_normalize(m, q0, psO[0], psO[1], qw=qw)
                    tail.append(o_mm)
                    drain(2)
                    if jt >= 2:
                        for _ in range(pops):
                            if filler:
                                filler.pop(0)()

            def y_unit(tt):
                def go():
                    ysb = work.tile([P, D], f16, tag="ysb", name="ysb",
                                    bufs=3)
                    psY = big_ps("psY")
                    for ncol in range(2):
                        for m in range(2):
                            nc.tensor.matmul(
                                psY[:, ncol * IC:(ncol + 1) * IC],
                                otn[m][:, tt * P:(tt + 1) * P],
                                wo_t[m][:, ncol * IC:(ncol + 1) * IC],
                                start=(m == 0), stop=(m == 1))
                    nc.vector.tensor_copy(ysb[:], psY[:])
                    nc.sync.dma_start(
                        out=ybounce[tt * P:(tt + 1) * P, :], in_=ysb[:])
                return go

            def y_pair_unit(tt):
                # two token tiles, one bounce DMA: shortens the last RS
                # piece's trigger path
                def go():
                    ysb = work.tile([P, 2, D], f16, tag="ysb2", name="ysb2",
                                    bufs=1)
                    for j in range(2):
                        psY = big_ps("psY")
                        for ncol in range(2):
                            for m in range(2):
                                nc.tensor.matmul(
                                    psY[:, ncol * IC:(ncol + 1) * IC],
                                    otn[m][:, (tt + j) * P:(tt + j + 1) * P],
                                    wo_t[m][:, ncol * IC:(ncol + 1) * IC],
                                    start=(m == 0), stop=(m == 1))
                        nc.vector.tensor_copy(ysb[:, j], psY[:])
                    nc.sync.dma_start(
                        out=ybounce[tt * P:(tt + 2) * P, :]
                        .rearrange("(j p) c -> p j c", j=2),
                        in_=ysb[:])
                return go

            def rs_unit(p):
                def go():
                    nc.gpsimd.collective_compute(
                        "ReduceScatter",
                        mybir.AluOpType.add,
                        replica_groups=groups,
                        ins=[ybounce[RS_START[p]:RS_START[p] + RS_ROWS[p], :]],
                        outs=[rs_out[p][:]],
                    )
                    # drain on the gpsimd queue: it blocks behind the
                    # collective anyway, so this issues right after RS p
                    # completes without head-blocking the sync queue
                    nc.gpsimd.dma_start(out=out_d[p][:], in_=rs_out[p][:])
                return go

            # ---- program ----
            # Warm the CC path during the input-DMA ramp.
            warm_in = dram.tile([4, D], f16, name="warm_in")
            warm_out = dram.tile([1, D], f16, name="warm_out")
            nc.gpsimd.collective_compute(
                "ReduceScatter", mybir.AluOpType.add, replica_groups=groups,
                ins=[warm_in[:]], outs=[warm_out[:]])

            with tc.tile_pool(name="qkv_in", bufs=1) as qkv_in:
                xt = []
                wq_t, wk_t, wv_t = [], [], []
                wqkv = qkv_in.tile([P, KD, 3 * DHC], bf16, name="wqkv")
                for k in range(KD):
                    xt.append(qkv_in.tile([P, T], bf16, name=f"xt{k}"))
                    wq_t.append(wqkv[:, k, 0:DHC])
                    wk_t.append(wqkv[:, k, DHC:2 * DHC])
                    wv_t.append(wqkv[:, k, 2 * DHC:3 * DHC])
                # column-blocked loads: everything the first qtkt/v units
                # need (wq|wk, x cols 0:512, wv) lands in the first ~2.5MB,
                # persist tensors (cmask/ones/wo) before the 3MB x tail.
                # k-pairs merged per DMA to halve the ~650ns/DMA issue cost.
                for k in range(0, KD, 2):
                    nc.sync.dma_start(
                        out=wqkv[:, k:k + 2, 0:2 * DHC],
                        in_=wqkv_d[k * P:(k + 2) * P, 0:2 * DHC]
                        .rearrange("(j p) c -> p j c", j=2))
                    nc.sync.dma_start(out=xt[k][:, 0:IC],
                                      in_=xT_d[k * P:(k + 1) * P, 0:IC])
                    nc.sync.dma_start(out=xt[k + 1][:, 0:IC],
                                      in_=xT_d[(k + 1) * P:(k + 2) * P, 0:IC])
                emit_early_persist_dmas()
                for k in range(0, KD, 2):
                    nc.sync.dma_start(
                        out=wqkv[:, k:k + 2, 2 * DHC:3 * DHC],
                        in_=wqkv_d[k * P:(k + 2) * P, 2 * DHC:3 * DHC]
                        .rearrange("(j p) c -> p j c", j=2))
                emit_wo_dma()
                for k in range(KD):
                    nc.sync.dma_start(out=xt[k][:, IC:T],
                                      in_=xT_d[k * P:(k + 1) * P, IC:T])
                ones_fill()

                # preload the ScalarE EXP table off the critical path (the
                # first ACTIVATE otherwise pays a ~1.3us ACT_TABLE_LOAD at
                # the start of attention)
                warm_act = work.tile([1, 2], f32, name="warm_act")
                nc.vector.memset(warm_act[:], 0.0)
                nc.scalar.activation(warm_act[:], warm_act[:],
                                     mybir.ActivationFunctionType.Exp,
                                     scale=SCALE)

                qtkt_unit(0, 0, wq_t, wk_t, xt)()
                qtkt_unit(1, 0, wq_t, wk_t, xt)()
                for tt in range(4):
                    v_unit(tt, wv_t, xt)()
                fillers = {
                    0: [qtkt_unit(0, 1, wq_t, wk_t, xt),
                        qtkt_unit(1, 1, wq_t, wk_t, xt)]
                       + [v_unit(tt, wv_t, xt) for tt in range(4, 8)],
                    1: [y_unit(tt) for tt in range(0, 4)] + [rs_unit(0)]
                       + [qtkt_unit(0, 2, wq_t, wk_t, xt),
                          qtkt_unit(1, 2, wq_t, wk_t, xt)]
                       + [v_unit(tt, wv_t, xt) for tt in range(8, 12)],
                    2: [y_unit(tt) for tt in range(4, 8)] + [rs_unit(1)]
                       + [qtkt_unit(0, 3, wq_t, wk_t, xt),
                          qtkt_unit(1, 3, wq_t, wk_t, xt)]
                       + [v_unit(tt, wv_t, xt) for tt in range(12, 16)],
                    3: [y_unit(tt) for tt in range(8, 12)] + [rs_unit(2)],
                }
                for c in range(NIC - 1):
                    f = fillers[c]
                    emit_qrange(c * IC, IC, 0, f, pops=2)
                    emit_qrange(c * IC, IC, 1, f, pops=2)
                    drain(0)
                    for g in f:
                        g()
                    f.clear()
                # last chunk as two 256-wide passes: rows 1536..1792 finish
                # ~25us before the end, so their RS piece overlaps the
                # second pass and only a 256-row piece is exposed at the tail
                f = fillers[NIC - 1]
                emit_qrange(3 * IC, IC // 2, 0, f, pops=2)
                emit_qrange(3 * IC, IC // 2, 1, f, pops=2)
                drain(0)
                for g in f:
                    g()
                f.clear()
                fl = [y_unit(12), y_unit(13), rs_unit(3)]
                emit_qrange(3 * IC + IC // 2, IC // 2, 0, fl, pops=1)
                emit_qrange(3 * IC + IC // 2, IC // 2, 1, fl, pops=1)
                drain(0)
                for g in fl:
                    g()
            for f in [y_pair_unit(14), rs_unit(4)]:
                f()

    nc.compile()
    return nc


def make_cmask():
    """[128, 128] triangular diagonal-block mask: key j (partition), query i
    (free col, relative to the key tile start): valid (0.0) iff j <= i."""
    j = np.arange(P)[:, None]
    i = np.arange(P)[None, :]
    return np.where(j <= i, 0.0, MASKVAL).astype(np.float32)


def shard_inputs(x, Wq, Wk, Wv, Wo):
    import ml_dtypes
    bf = ml_dtypes.bfloat16
    cmask = make_cmask()
    in_maps = []
    for c in range(N_CORES):
        b, r = CORE_B[c], CORE_R[c]
        sl = slice(r * DHC, (r + 1) * DHC)
        wqkv = np.concatenate([Wq[:, sl], Wk[:, sl], Wv[:, sl]], axis=1)
        in_maps.append({
            "xT": np.ascontiguousarray(x[b].T).astype(bf),
            "wqkv": np.ascontiguousarray(wqkv).astype(bf),
            "wo": np.ascontiguousarray(Wo[sl, :]).astype(bf),
            "cmask": cmask,
            "ones": np.ones((P, DH), dtype=bf),
        })
    return in_maps


def assemble(results, B=2):
    out = np.empty((B, T, D), dtype=np.float32)
    for c in range(N_CORES):
        b, r = CORE_B[c], CORE_R[c]
        for p in range(NP_RS):
            res = results[c][f"out{p}"].astype(np.float32)
            rows = RS_ROWS[p] // 4
            out[b, RS_START[p] + r * rows: RS_START[p] + (r + 1) * rows, :] \
                = res
    return out


_NC_CACHE = None


def get_nc():
    global _NC_CACHE
    if _NC_CACHE is None:
        _NC_CACHE = build_nc()
    return _NC_CACHE


def run(inputs, trace=False):
    nc = get_nc()
    in_maps = shard_inputs(inputs["x"], inputs["Wq"], inputs["Wk"],
                           inputs["Wv"], inputs["Wo"])
    res = run_bass_kernel_spmd(nc, in_maps, core_ids=list(range(N_CORES)),
                               trace=trace)
    return assemble(res.results), res


def kernel(x, Wq, Wk, Wv, Wo):
    out, _ = run({"x": np.asarray(x), "Wq": np.asarray(Wq),
                  "Wk": np.asarray(Wk), "Wv": np.asarray(Wv),
                  "Wo": np.asarray(Wo)})
    return out
